# revision 5
# baseline (speedup 1.0000x reference)
"""Trainium2 Bass kernel: ConvolutionalMultiheadAttention.

Reference computation (per batch element b):
    q = conv1d(x, w0) + b0          # [D, Lp]  (VALID, K=3)
    k = conv1d(x, w1) + b1
    v = conv1d(x, w2) + b2
    per head h (Dh=64): out_h = v_h @ softmax(q_h^T k_h / sqrt(D))^T

Sharding: data-parallel over batch B=16 across 8 cores (2 per core).
Weights replicated. No collectives.

Per-core kernel architecture:
  - conv as matmul: contraction over input channel i (4 chunks of 128),
    accumulating 4*3 = 12 matmuls per PSUM tile. q,k produced in
    [o_part, t_free] layout; v produced transposed [t_part, o_free]
    (lhsT = x slice, rhs = w2 slice) with a constant 1.0 column appended
    per head (65-wide) so the attention AV matmul also yields the
    softmax denominator row.
  - scores computed transposed: S_T[kt, qt] = k_h^T q_h (contraction
    over d=64 on partitions). exp via ACT engine with the 1/sqrt(512)
    scale folded into the activation, output in fp16 (P_T).
  - AV: out[65, qt] = [v_h | 1]^T @ P_T accumulated over kt chunks.
    Row 64 is the softmax denominator. Normalize: reciprocal (DVE) +
    partition_broadcast (GPSIMD) + multiply (DVE), then DMA straight to
    the output in [o, t] layout.
  - fp32r matmuls (full PE rate at N>=256) for convs/scores; fp16 for
    the P_T/V attention matmul (P in [0, e^4], fp16 rel err ~5e-4).
  - input DMAs split per chunk so the first conv matmuls start ~3.5us
    in; conv of batch b=1 is interleaved into the attention pair loop
    of b=0 so the PE has work while ACT streams exps.
"""

import numpy as np

import concourse.bass as bass
import concourse.bacc as bacc
import concourse.mybir as mybir
import concourse.tile as tile
from concourse.bass_utils import run_bass_kernel_spmd

B, D, L, KW, H = 16, 512, 1024, 3, 8
LP = L - KW + 1          # 1022
DH = D // H              # 64
NCORES = 8
BLOC = B // NCORES       # 2
NIC = D // 128           # 4 input-channel chunks
SCALE = 1.0 / float(np.sqrt(D))
IC_MAJOR = False
import os
MM_DTYPE_NAME = os.environ.get('MM_DTYPE', 'bf16')

F32 = mybir.dt.float32
F32R = mybir.dt.float32r
F16 = mybir.dt.float16
BF16 = mybir.dt.bfloat16
MMDT = {"f32r": F32R, "bf16": BF16, "f32": F32}[MM_DTYPE_NAME]
import ml_dtypes
MMDT_NP = {"f32r": np.float32, "bf16": ml_dtypes.bfloat16, "f32": np.float32}[MM_DTYPE_NAME]

# time chunking
TQ = [(0, 512), (512, LP - 512)]                       # qt chunks (512, 510)
TKC = [(i * 128, min(128, LP - i * 128)) for i in range(8)]  # kt chunks (...126)


def _emit(tc, xs, wq, wk, wv, bq, bk, bv, out, loop_n=None):
    nc = tc.nc
    Exp = mybir.ActivationFunctionType.Exp
    from concourse.alu_op_type import AluOpType
    Add = AluOpType.add
    from contextlib import ExitStack
    ctx = ExitStack()
    wpool = ctx.enter_context(tc.tile_pool(name="w", bufs=1))
    cpool = ctx.enter_context(tc.tile_pool(name="const", bufs=1))
    xpool = ctx.enter_context(tc.tile_pool(name="x", bufs=1))
    qkpool = ctx.enter_context(tc.tile_pool(name="qk", bufs=1))
    vpool = ctx.enter_context(tc.tile_pool(name="v", bufs=2))
    ptpool = ctx.enter_context(tc.tile_pool(name="pt", bufs=12))
    opool = ctx.enter_context(tc.tile_pool(name="o", bufs=3))
    rpool = ctx.enter_context(tc.tile_pool(name="r", bufs=2))
    bpool = ctx.enter_context(tc.tile_pool(name="bc", bufs=2))
    # PSUM pools are opened in two phases (static 8-bank budget):
    # phase 1: pconv8 (8 banks, b0 convs) — closed before phase 2
    # phase 2: pconv (2) + pscore (2x2) + pav (2)
    # Under loop_n (HW-timing loop), a single phase is used so no pool
    # opens/closes inside the For_i body.
    psum_pools = {}
    two_phase = IC_MAJOR and loop_n is None
    if not two_phase:
        psum_pools["pconv"] = ctx.enter_context(
            tc.tile_pool(name="pconv", bufs=2, space="PSUM"))
        psum_pools["pscore"] = ctx.enter_context(
            tc.tile_pool(name="pscore", bufs=2, space="PSUM"))
        psum_pools["pav"] = ctx.enter_context(
            tc.tile_pool(name="pav", bufs=2, space="PSUM"))

    if loop_n is not None:
        loop_cm = tc.For_i(0, loop_n, 1)
        loop_cm.__enter__()

    # ---- loads (split + ordered so the first conv matmuls start early
    # and each conv's weights land just before it needs them) ----
    wq_ic = []
    wk_ic = []
    x_t = [[None] * NIC for _ in range(BLOC)]
    for ic in range(NIC):
        t = wpool.tile([128, KW, D], MMDT, tag=f"wq{ic}", name=f"wq{ic}")
        if ic == 0:
            # split the gating tile so the first conv matmul (needs only
            # kk=0) starts as soon as a 128KB slice lands, not the full
            # 393KB tile
            for kk in range(KW):
                nc.sync.dma_start(t[:, kk], wq[:, ic, kk])
        else:
            nc.sync.dma_start(t[:], wq[:, ic])
        wq_ic.append(t)
        xt = xpool.tile([128, L], MMDT, tag=f"x0{ic}", name=f"x0{ic}")
        if ic == 0:
            nc.sync.dma_start(
                xt[:, 0:516],
                xs[0].rearrange("(c p) t -> p c t", p=128)[:, ic, 0:516])
            nc.sync.dma_start(
                xt[:, 516:L],
                xs[0].rearrange("(c p) t -> p c t", p=128)[:, ic, 516:L])
        else:
            nc.sync.dma_start(
                xt[:], xs[0].rearrange("(c p) t -> p c t", p=128)[:, ic])
        x_t[0][ic] = xt
        if ic == 0:
            bq_sb = cpool.tile([128, NIC], F32, tag="bq")
            nc.sync.dma_start(bq_sb[:], bq[:])
            bk_sb = cpool.tile([128, NIC], F32, tag="bk")
            nc.sync.dma_start(bk_sb[:], bk[:])
            bv_sb = cpool.tile([128, D], F32, tag="bv")
            nc.sync.dma_start(bv_sb[:], bv[:])
    for ic in range(NIC):
        t = wpool.tile([128, KW, D], MMDT, tag=f"wk{ic}", name=f"wk{ic}")
        nc.sync.dma_start(t[:], wk[:, ic])
        wk_ic.append(t)
    wv_sb = wpool.tile([128, NIC, KW, D], MMDT, tag="wv")
    nc.sync.dma_start(wv_sb[:], wv[:])
    for ic in range(NIC):
        xt = xpool.tile([128, L], MMDT, tag=f"x1{ic}", name=f"x1{ic}")
        nc.sync.dma_start(
            xt[:], xs[1].rearrange("(c p) t -> p c t", p=128)[:, ic])
        x_t[1][ic] = xt

    def w_slice(nm, ic, kk, osl):
        if nm == "q":
            return wq_ic[ic][:, kk, osl]
        if nm == "k":
            return wk_ic[ic][:, kk, osl]
        return wv_sb[:, ic, kk, osl]

    # q/k: per-oc tiles [p, t] with o = oc*128+p (reused in-place across b)
    q_oc = [qkpool.tile([128, L], MMDT, tag=f"q{oc}", name=f"q{oc}") for oc in range(NIC)]
    k_oc = [qkpool.tile([128, L], MMDT, tag=f"k{oc}", name=f"k{oc}") for oc in range(NIC)]
    # v: [p(t in chunk), ktc, h, 0:64] + ones col; double-buffered across b
    v_tiles = [None, None]

    def conv_qk_piece(b, nm, oc):
        dst = (q_oc if nm == "q" else k_oc)[oc]
        bias_sb = bq_sb if nm == "q" else bk_sb
        for (t0, tn) in TQ:
            ps = psum_pools["pconv"].tile([128, 512], F32, tag="pc", name="pc")
            mm = 0
            for ic in range(NIC):
                for kk in range(KW):
                    nc.tensor.matmul(
                        ps[:, :tn],
                        w_slice(nm, ic, kk, slice(oc * 128, (oc + 1) * 128)),
                        x_t[b][ic][:, t0 + kk:t0 + kk + tn],
                        start=(mm == 0), stop=(mm == NIC * KW - 1),
                    )
                    mm += 1
            nc.vector.tensor_tensor(
                dst[:, t0:t0 + tn], ps[:, :tn],
                bias_sb[:, oc:oc + 1].broadcast_to([128, tn]), op=Add,
            )

    def v_alloc(b):
        v_sb = vpool.tile([128, 8, H, DH + 1], F16, tag="v")
        nc.gpsimd.memset(v_sb[:, :, :, DH:DH + 1], 1.0)
        v_tiles[b] = v_sb

    def conv_v_piece(b, tci):
        t0, tn = TKC[tci]
        ps = psum_pools["pconv"].tile([128, 512], F32, tag="pc", name="pc")
        mm = 0
        for ic in range(NIC):
            for kk in range(KW):
                nc.tensor.matmul(
                    ps[:tn, :],
                    x_t[b][ic][:, t0 + kk:t0 + kk + tn],
                    wv_sb[:, ic, kk, :],
                    start=(mm == 0), stop=(mm == NIC * KW - 1),
                )
                mm += 1
        nc.vector.tensor_tensor(
            v_tiles[b][:tn, tci, :, 0:DH],
            ps[:tn].rearrange("p (h d) -> p h d", h=H),
            bv_sb[:tn].rearrange("p (h d) -> p h d", h=H),
            op=Add,
        )

    # ---- attention ----
    pt_tiles = {}

    def scores(b, h):
        po = 64 * (h % 2)
        oc = h // 2
        kh = k_oc[oc][po:po + DH, :]
        qh = q_oc[oc][po:po + DH, :]
        tiles = []
        for (kt0, ktn) in TKC:
            pt = ptpool.tile([128, L], F16, tag="pt")
            ss = psum_pools["pscore"].tile([128, 1024], F32, tag="ps", name="ss")
            for (qt0, qtn) in TQ:
                nc.tensor.matmul(
                    ss[:ktn, qt0:qt0 + qtn],
                    kh[:, kt0:kt0 + ktn],
                    qh[:, qt0:qt0 + qtn],
                    start=True, stop=True,
                )
            nc.scalar.activation(pt[:ktn, 0:LP], ss[:ktn, 0:LP], Exp,
                                 scale=SCALE)
            tiles.append(pt)
        pt_tiles[(b, h)] = tiles

    def av(b, h):
        tiles = pt_tiles.pop((b, h))
        for (qt0, qtn) in TQ:
            pa = psum_pools["pav"].tile([128, 512], F32, tag="pa", name="pa")
            for tci, (kt0, ktn) in enumerate(TKC):
                nc.tensor.matmul(
                    pa[:DH + 1, :qtn],
                    v_tiles[b][:ktn, tci, h, :],
                    tiles[tci][:ktn, qt0:qt0 + qtn],
                    start=(tci == 0), stop=(tci == len(TKC) - 1),
                )
            rec = rpool.tile([1, 512], F32, tag="rec")
            nc.vector.reciprocal(rec[:1, :qtn], pa[DH:DH + 1, :qtn])
            brd = bpool.tile([DH, 512], F32, tag="brd")
            nc.gpsimd.partition_broadcast(brd[:, :qtn], rec[:1, :qtn])
            ot = opool.tile([DH, 512], F32, tag="ot")
            nc.vector.tensor_mul(ot[:, :qtn], pa[0:DH, :qtn], brd[:, :qtn])
            nc.sync.dma_start(
                out[b, DH * h:DH * (h + 1), qt0:qt0 + qtn], ot[:, :qtn]
            )

    # conv b=0: with an 8-bank scoped PSUM pool (closed before the
    # attention PSUM pools open — PSUM pools reserve banks statically).
    def conv_b0_with_pool(pconv8):

        def conv_qk_b0_icmajor(nm):
            dst_l = q_oc if nm == "q" else k_oc
            bias_sb = bq_sb if nm == "q" else bk_sb
            groups = [(oc, t0, tn) for oc in range(NIC) for (t0, tn) in TQ]
            tiles = [pconv8.tile([128, 512], F32, tag="pc8",
                                 name=f"pc8_{nm}{gi}")
                     for gi in range(len(groups))]
            for ic in range(NIC):
                for kk in range(KW):
                    for gi, (oc, t0, tn) in enumerate(groups):
                        nc.tensor.matmul(
                            tiles[gi][:, :tn],
                            w_slice(nm, ic, kk, slice(oc * 128, (oc + 1) * 128)),
                            x_t[0][ic][:, t0 + kk:t0 + kk + tn],
                            start=(ic == 0 and kk == 0),
                            stop=(ic == NIC - 1 and kk == KW - 1),
                        )
            for gi, (oc, t0, tn) in enumerate(groups):
                nc.vector.tensor_tensor(
                    dst_l[oc][:, t0:t0 + tn], tiles[gi][:, :tn],
                    bias_sb[:, oc:oc + 1].broadcast_to([128, tn]), op=Add,
                )

        def conv_v_b0_icmajor():
            tiles = [pconv8.tile([128, 512], F32, tag="pc8",
                                 name=f"pc8_v{gi}")
                     for gi in range(len(TKC))]
            for ic in range(NIC):
                for kk in range(KW):
                    for gi, (t0, tn) in enumerate(TKC):
                        nc.tensor.matmul(
                            tiles[gi][:tn, :],
                            x_t[0][ic][:, t0 + kk:t0 + kk + tn],
                            wv_sb[:, ic, kk, :],
                            start=(ic == 0 and kk == 0),
                            stop=(ic == NIC - 1 and kk == KW - 1),
                        )
            for gi, (t0, tn) in enumerate(TKC):
                nc.vector.tensor_tensor(
                    v_tiles[0][:tn, gi, :, 0:DH],
                    tiles[gi][:tn].rearrange("p (h d) -> p h d", h=H),
                    bv_sb[:tn].rearrange("p (h d) -> p h d", h=H),
                    op=Add,
                )

        if IC_MAJOR:
            conv_qk_b0_icmajor("q")
            conv_qk_b0_icmajor("k")
            v_alloc(0)
            conv_v_b0_icmajor()
        else:
            groups = [(oc, t0, tn) for oc in range(NIC) for (t0, tn) in TQ]
            for nm in ("q", "k"):
                dst_l = q_oc if nm == "q" else k_oc
                bias_sb = bq_sb if nm == "q" else bk_sb
                for (oc, t0, tn) in groups:
                    ps = pconv8.tile([128, 512], F32, tag="pc8", name="pc8")
                    mm = 0
                    for ic in range(NIC):
                        for kk in range(KW):
                            nc.tensor.matmul(
                                ps[:, :tn],
                                w_slice(nm, ic, kk,
                                        slice(oc * 128, (oc + 1) * 128)),
                                x_t[0][ic][:, t0 + kk:t0 + kk + tn],
                                start=(mm == 0), stop=(mm == NIC * KW - 1),
                            )
                            mm += 1
                    nc.vector.tensor_tensor(
                        dst_l[oc][:, t0:t0 + tn], ps[:, :tn],
                        bias_sb[:, oc:oc + 1].broadcast_to([128, tn]), op=Add,
                    )
            v_alloc(0)
            for gi, (t0, tn) in enumerate(TKC):
                ps = pconv8.tile([128, 512], F32, tag="pc8", name="pc8")
                mm = 0
                for ic in range(NIC):
                    for kk in range(KW):
                        nc.tensor.matmul(
                            ps[:tn, :],
                            x_t[0][ic][:, t0 + kk:t0 + kk + tn],
                            wv_sb[:, ic, kk, :],
                            start=(mm == 0), stop=(mm == NIC * KW - 1),
                        )
                        mm += 1
                nc.vector.tensor_tensor(
                    v_tiles[0][:tn, gi, :, 0:DH],
                    ps[:tn].rearrange("p (h d) -> p h d", h=H),
                    bv_sb[:tn].rearrange("p (h d) -> p h d", h=H),
                    op=Add,
                )

    if two_phase:
        with tc.tile_pool(name="pconv8", bufs=8, space="PSUM") as pconv8:
            conv_b0_with_pool(pconv8)
        psum_pools["pconv"] = ctx.enter_context(
            tc.tile_pool(name="pconv", bufs=2, space="PSUM"))
        psum_pools["pscore"] = ctx.enter_context(
            tc.tile_pool(name="pscore", bufs=2, space="PSUM"))
        psum_pools["pav"] = ctx.enter_context(
            tc.tile_pool(name="pav", bufs=2, space="PSUM"))
    else:
        for oc in range(NIC):
            conv_qk_piece(0, "q", oc)
        for oc in range(NIC):
            conv_qk_piece(0, "k", oc)
        v_alloc(0)
        for tci in range(len(TKC)):
            conv_v_piece(0, tci)

    # attention b=0 with conv b=1 injected between pairs (fills PE while
    # the ACT engine streams exps; evictions wait on b=0 reads per-tile)
    def inject(h):
        if h == 0:
            v_alloc(1)
            for tci in range(4):
                conv_v_piece(1, tci)
        elif h == 1:
            for tci in range(4, 8):
                conv_v_piece(1, tci)
        elif h in (2, 3, 4):
            # q_oc[oc]/k_oc[oc] are read by scores(0, 2oc) and scores(0, 2oc+1);
            # scores(0, j) is emitted at h = j-1, so conv(1, oc) may only be
            # emitted at h >= 2oc (oc=2 lands exactly at its boundary).
            oc = h - 2
            conv_qk_piece(1, "q", oc)
            conv_qk_piece(1, "k", oc)
        elif h == 5:
            # pull b1's first score pair forward so ACT has exp work
            # queued before the conv filler runs out
            scores(1, 0)
        elif h == 6:
            conv_qk_piece(1, "q", 3)
            conv_qk_piece(1, "k", 3)
            scores(1, 1)

    scores(0, 0)
    for h in range(H):
        if h + 1 < H:
            scores(0, h + 1)
        av(0, h)
        inject(h)

    for h in range(H):
        if h + 1 < H and (1, h + 1) not in pt_tiles:
            scores(1, h + 1)
        av(1, h)

    if loop_n is not None:
        loop_cm.__exit__(None, None, None)
    ctx.close()


_CACHE = {}


def _build(loop_n=None):
    key = ("nc", loop_n)
    if key in _CACHE:
        return _CACHE[key]
    nc = bacc.Bacc("TRN2", target_bir_lowering=False, debug=False,
                   num_devices=NCORES)
    xs = nc.dram_tensor("xs", [BLOC, D, L], MMDT, kind="ExternalInput").ap()
    wq = nc.dram_tensor("wqt", [128, NIC, KW, D], MMDT, kind="ExternalInput").ap()
    wk = nc.dram_tensor("wkt", [128, NIC, KW, D], MMDT, kind="ExternalInput").ap()
    wv = nc.dram_tensor("wvt", [128, NIC, KW, D], MMDT, kind="ExternalInput").ap()
    bq = nc.dram_tensor("bq", [128, NIC], F32, kind="ExternalInput").ap()
    bk = nc.dram_tensor("bk", [128, NIC], F32, kind="ExternalInput").ap()
    bv = nc.dram_tensor("bv", [128, D], F32, kind="ExternalInput").ap()
    out = nc.dram_tensor("out", [BLOC, D, LP], F32, kind="ExternalOutput").ap()
    with tile.TileContext(nc) as tc:
        _emit(tc, xs, wq, wk, wv, bq, bk, bv, out, loop_n=loop_n)
    nc.compile()
    _CACHE[key] = nc
    return nc


def _wt(w):
    # w: [O, I, K] -> [p, ic, k, o] with i = ic*128 + p
    return np.ascontiguousarray(
        w.transpose(1, 2, 0).reshape(NIC, 128, KW, D).transpose(1, 0, 2, 3)
    ).astype(MMDT_NP)


def _in_common(w0, b0, w1, b1, w2, b2):
    return {
        "wqt": _wt(np.asarray(w0, np.float32)),
        "wkt": _wt(np.asarray(w1, np.float32)),
        "wvt": _wt(np.asarray(w2, np.float32)),
        # per-partition bias layouts: [p, oc] with o = oc*128+p
        "bq": np.ascontiguousarray(
            np.asarray(b0, np.float32).reshape(NIC, 128).T),
        "bk": np.ascontiguousarray(
            np.asarray(b1, np.float32).reshape(NIC, 128).T),
        "bv": np.ascontiguousarray(
            np.tile(np.asarray(b2, np.float32)[None, :], (128, 1))),
    }


def kernel(x, w0, b0, w1, b1, w2, b2):
    x = np.asarray(x, dtype=np.float32)
    inp_common = _in_common(w0, b0, w1, b1, w2, b2)
    nc = _build()
    in_maps = [
        {"xs": np.ascontiguousarray(x[c * BLOC:(c + 1) * BLOC]).astype(MMDT_NP), **inp_common}
        for c in range(NCORES)
    ]
    res = run_bass_kernel_spmd(nc, in_maps, list(range(NCORES)))
    return np.concatenate([res.results[c]["out"] for c in range(NCORES)], axis=0)


def run_traced(x, w0, b0, w1, b1, w2, b2, **kw):
    """Like kernel() but returns (output, BassKernelResults)."""
    x = np.asarray(x, dtype=np.float32)
    inp_common = _in_common(w0, b0, w1, b1, w2, b2)
    nc = _build()
    in_maps = [
        {"xs": np.ascontiguousarray(x[c * BLOC:(c + 1) * BLOC]).astype(MMDT_NP), **inp_common}
        for c in range(NCORES)
    ]
    res = run_bass_kernel_spmd(nc, in_maps, list(range(NCORES)), **kw)
    out = np.concatenate([res.results[c]["out"] for c in range(NCORES)], axis=0)
    return out, res



# revision 12
# speedup vs baseline: 1.2036x; 1.2036x over previous
"""Trainium2 Bass kernel: ConvolutionalMultiheadAttention.

Reference computation (per batch element b):
    q = conv1d(x, w0) + b0          # [D, Lp]  (VALID, K=3)
    k = conv1d(x, w1) + b1
    v = conv1d(x, w2) + b2
    per head h (Dh=64): out_h = v_h @ softmax(q_h^T k_h / sqrt(D))^T

Sharding: data-parallel over batch B=16 across 8 cores (2 per core).
Weights replicated. No collectives.

Per-core kernel architecture:
  - conv as matmul: contraction over input channel i (4 chunks of 128),
    accumulating 4*3 = 12 matmuls per PSUM tile. q,k produced in
    [o_part, t_free] layout; v produced transposed [t_part, o_free]
    (lhsT = x slice, rhs = w2 slice) with a constant 1.0 column appended
    per head (65-wide) so the attention AV matmul also yields the
    softmax denominator row.
  - scores computed transposed: S_T[kt, qt] = k_h^T q_h (contraction
    over d=64 on partitions). exp via ACT engine with the 1/sqrt(512)
    scale folded into the activation, output in fp16 (P_T).
  - AV: out[65, qt] = [v_h | 1]^T @ P_T accumulated over kt chunks.
    Row 64 is the softmax denominator. Normalize: reciprocal (DVE) +
    partition_broadcast (GPSIMD) + multiply (DVE), then DMA straight to
    the output in [o, t] layout.
  - fp32r matmuls (full PE rate at N>=256) for convs/scores; fp16 for
    the P_T/V attention matmul (P in [0, e^4], fp16 rel err ~5e-4).
  - input DMAs split per chunk so the first conv matmuls start ~3.5us
    in; conv of batch b=1 is interleaved into the attention pair loop
    of b=0 so the PE has work while ACT streams exps.
"""

import numpy as np

import concourse.bass as bass
import concourse.bacc as bacc
import concourse.mybir as mybir
import concourse.tile as tile
from concourse.bass_utils import run_bass_kernel_spmd

B, D, L, KW, H = 16, 512, 1024, 3, 8
LP = L - KW + 1          # 1022
DH = D // H              # 64
NCORES = 8
BLOC = B // NCORES       # 2
NIC = D // 128           # 4 input-channel chunks
SCALE = 1.0 / float(np.sqrt(D))
IC_MAJOR = False
import os
MM_DTYPE_NAME = os.environ.get('MM_DTYPE', 'bf16')

F32 = mybir.dt.float32
F32R = mybir.dt.float32r
F16 = mybir.dt.float16
BF16 = mybir.dt.bfloat16
MMDT = {"f32r": F32R, "bf16": BF16, "f32": F32}[MM_DTYPE_NAME]
import ml_dtypes
MMDT_NP = {"f32r": np.float32, "bf16": ml_dtypes.bfloat16, "f32": np.float32}[MM_DTYPE_NAME]

# time chunking
TQ = [(0, 512), (512, LP - 512)]                       # qt chunks (512, 510)
TKC = [(i * 128, min(128, LP - i * 128)) for i in range(8)]  # kt chunks (...126)


def _emit(tc, xs, wq, wk, wv, bq, bk, bv, out, loop_n=None):
    nc = tc.nc
    Exp = mybir.ActivationFunctionType.Exp
    from concourse.alu_op_type import AluOpType
    Add = AluOpType.add
    from contextlib import ExitStack
    ctx = ExitStack()
    wpool = ctx.enter_context(tc.tile_pool(name="w", bufs=1))
    cpool = ctx.enter_context(tc.tile_pool(name="const", bufs=1))
    xpool = ctx.enter_context(tc.tile_pool(name="x", bufs=1))
    qkpool = ctx.enter_context(tc.tile_pool(name="qk", bufs=1))
    vpool = ctx.enter_context(tc.tile_pool(name="v", bufs=2))
    ptpool = ctx.enter_context(tc.tile_pool(name="pt", bufs=12))
    opool = ctx.enter_context(tc.tile_pool(name="o", bufs=3))
    rpool = ctx.enter_context(tc.tile_pool(name="r", bufs=2))
    bpool = ctx.enter_context(tc.tile_pool(name="bc", bufs=2))
    # PSUM pools are opened in two phases (static 8-bank budget):
    # phase 1: pconv8 (8 banks, b0 convs) — closed before phase 2
    # phase 2: pconv (2) + pscore (2x2) + pav (2)
    # Under loop_n (HW-timing loop), a single phase is used so no pool
    # opens/closes inside the For_i body.
    psum_pools = {}
    two_phase = IC_MAJOR and loop_n is None
    if not two_phase:
        psum_pools["pconv"] = ctx.enter_context(
            tc.tile_pool(name="pconv", bufs=2, space="PSUM"))
        psum_pools["pscore"] = ctx.enter_context(
            tc.tile_pool(name="pscore", bufs=2, space="PSUM"))
        psum_pools["pav"] = ctx.enter_context(
            tc.tile_pool(name="pav", bufs=2, space="PSUM"))

    if loop_n is not None:
        loop_cm = tc.For_i(0, loop_n, 1)
        loop_cm.__enter__()

    # ---- loads (split + ordered so the first conv matmuls start early
    # and each conv's weights land just before it needs them) ----
    wq_ic = []
    wk_ic = []
    x_t = [[None] * NIC for _ in range(BLOC)]
    for ic in range(NIC):
        t = wpool.tile([128, KW, D], MMDT, tag=f"wq{ic}", name=f"wq{ic}")
        if ic == 0:
            # split the gating tile so the first conv matmul (needs only
            # kk=0) starts as soon as a 128KB slice lands, not the full
            # 393KB tile
            for kk in range(KW):
                nc.sync.dma_start(t[:, kk], wq[:, ic, kk])
        else:
            nc.sync.dma_start(t[:], wq[:, ic])
        wq_ic.append(t)
        xt = xpool.tile([128, L], MMDT, tag=f"x0{ic}", name=f"x0{ic}")
        if ic == 0:
            nc.sync.dma_start(
                xt[:, 0:516],
                xs[0].rearrange("(c p) t -> p c t", p=128)[:, ic, 0:516])
            nc.sync.dma_start(
                xt[:, 516:L],
                xs[0].rearrange("(c p) t -> p c t", p=128)[:, ic, 516:L])
        else:
            nc.sync.dma_start(
                xt[:], xs[0].rearrange("(c p) t -> p c t", p=128)[:, ic])
        x_t[0][ic] = xt
        if ic == 0:
            bq_sb = cpool.tile([128, NIC], F32, tag="bq")
            nc.sync.dma_start(bq_sb[:], bq[:])
            bk_sb = cpool.tile([128, NIC], F32, tag="bk")
            nc.sync.dma_start(bk_sb[:], bk[:])
            bv_sb = cpool.tile([128, D], F32, tag="bv")
            nc.sync.dma_start(bv_sb[:], bv[:])
    for ic in range(NIC):
        t = wpool.tile([128, KW, D], MMDT, tag=f"wk{ic}", name=f"wk{ic}")
        nc.sync.dma_start(t[:], wk[:, ic])
        wk_ic.append(t)
    wv_sb = wpool.tile([128, NIC, KW, D], MMDT, tag="wv")
    nc.sync.dma_start(wv_sb[:], wv[:])
    for ic in range(NIC):
        xt = xpool.tile([128, L], MMDT, tag=f"x1{ic}", name=f"x1{ic}")
        nc.sync.dma_start(
            xt[:], xs[1].rearrange("(c p) t -> p c t", p=128)[:, ic])
        x_t[1][ic] = xt

    def w_slice(nm, ic, kk, osl):
        if nm == "q":
            return wq_ic[ic][:, kk, osl]
        if nm == "k":
            return wk_ic[ic][:, kk, osl]
        return wv_sb[:, ic, kk, osl]

    # q/k: per-oc tiles [p, t] with o = oc*128+p (reused in-place across b)
    q_oc = [qkpool.tile([128, L], MMDT, tag=f"q{oc}", name=f"q{oc}") for oc in range(NIC)]
    k_oc = [qkpool.tile([128, L], MMDT, tag=f"k{oc}", name=f"k{oc}") for oc in range(NIC)]
    # v: [p(t in chunk), ktc, h, 0:64] + ones col; double-buffered across b
    v_tiles = [None, None]

    def conv_qk_piece(b, nm, oc):
        dst = (q_oc if nm == "q" else k_oc)[oc]
        bias_sb = bq_sb if nm == "q" else bk_sb
        for (t0, tn) in TQ:
            ps = psum_pools["pconv"].tile([128, 512], F32, tag="pc", name="pc")
            mm = 0
            for ic in range(NIC):
                for kk in range(KW):
                    nc.tensor.matmul(
                        ps[:, :tn],
                        w_slice(nm, ic, kk, slice(oc * 128, (oc + 1) * 128)),
                        x_t[b][ic][:, t0 + kk:t0 + kk + tn],
                        start=(mm == 0), stop=(mm == NIC * KW - 1),
                    )
                    mm += 1
            nc.vector.tensor_tensor(
                dst[:, t0:t0 + tn], ps[:, :tn],
                bias_sb[:, oc:oc + 1].broadcast_to([128, tn]), op=Add,
            )

    # ones in columns 0..63, v in columns 64..127: the AV matmul then puts
    # the softmax denominator on PSUM partition 0 (reciprocal_approx_fast
    # requires a zero base partition; PSUM reads must be 0/64-aligned, so
    # the out block sits at partitions 64..127)
    def v_alloc(b):
        v_sb = vpool.tile([128, 8, H, 2 * DH], F16, tag="v")
        nc.vector.memset(v_sb[:, :, :, 0:DH], 1.0)
        v_tiles[b] = v_sb

    def conv_v_piece(b, tci):
        t0, tn = TKC[tci]
        ps = psum_pools["pconv"].tile([128, 512], F32, tag="pc", name="pc")
        mm = 0
        for ic in range(NIC):
            for kk in range(KW):
                nc.tensor.matmul(
                    ps[:tn, :],
                    x_t[b][ic][:, t0 + kk:t0 + kk + tn],
                    wv_sb[:, ic, kk, :],
                    start=(mm == 0), stop=(mm == NIC * KW - 1),
                )
                mm += 1
        nc.vector.tensor_tensor(
            v_tiles[b][:tn, tci, :, DH:2 * DH],
            ps[:tn].rearrange("p (h d) -> p h d", h=H),
            bv_sb[:tn].rearrange("p (h d) -> p h d", h=H),
            op=Add,
        )

    # ---- attention ----
    pt_tiles = {}

    def scores(b, h):
        po = 64 * (h % 2)
        oc = h // 2
        kh = k_oc[oc][po:po + DH, :]
        qh = q_oc[oc][po:po + DH, :]
        tiles = []
        for (kt0, ktn) in TKC:
            pt = ptpool.tile([128, L], F16, tag="pt")
            ss = psum_pools["pscore"].tile([128, 1024], F32, tag="ps", name="ss")
            for (qt0, qtn) in TQ:
                nc.tensor.matmul(
                    ss[:ktn, qt0:qt0 + qtn],
                    kh[:, kt0:kt0 + ktn],
                    qh[:, qt0:qt0 + qtn],
                    start=True, stop=True,
                )
            nc.scalar.activation(pt[:ktn, 0:LP], ss[:ktn, 0:LP], Exp,
                                 scale=SCALE)
            tiles.append(pt)
        pt_tiles[(b, h)] = tiles

    def av(b, h):
        tiles = pt_tiles.pop((b, h))
        for (qt0, qtn) in TQ:
            pa = psum_pools["pav"].tile([128, 512], F32, tag="pa", name="pa")
            for tci, (kt0, ktn) in enumerate(TKC):
                nc.tensor.matmul(
                    pa[:2 * DH, :qtn],
                    v_tiles[b][:ktn, tci, h, :],
                    tiles[tci][:ktn, qt0:qt0 + qtn],
                    start=(tci == 0), stop=(tci == len(TKC) - 1),
                )
            rec = rpool.tile([1, 512], F32, tag="rec")
            nc.vector.reciprocal_approx_fast(rec[:1, :qtn], pa[0:1, :qtn])
            brd = bpool.tile([DH, 512], F32, tag="brd")
            nc.gpsimd.partition_broadcast(brd[:, :qtn], rec[:1, :qtn])
            ot = opool.tile([DH, 512], F32, tag="ot")
            nc.vector.tensor_mul(ot[:, :qtn], pa[DH:2 * DH, :qtn], brd[:, :qtn])
            nc.sync.dma_start(
                out[b, DH * h:DH * (h + 1), qt0:qt0 + qtn], ot[:, :qtn]
            )

    # conv b=0: with an 8-bank scoped PSUM pool (closed before the
    # attention PSUM pools open — PSUM pools reserve banks statically).
    def conv_b0_with_pool(pconv8):

        def conv_qk_b0_icmajor(nm):
            dst_l = q_oc if nm == "q" else k_oc
            bias_sb = bq_sb if nm == "q" else bk_sb
            groups = [(oc, t0, tn) for oc in range(NIC) for (t0, tn) in TQ]
            tiles = [pconv8.tile([128, 512], F32, tag="pc8",
                                 name=f"pc8_{nm}{gi}")
                     for gi in range(len(groups))]
            for ic in range(NIC):
                for kk in range(KW):
                    for gi, (oc, t0, tn) in enumerate(groups):
                        nc.tensor.matmul(
                            tiles[gi][:, :tn],
                            w_slice(nm, ic, kk, slice(oc * 128, (oc + 1) * 128)),
                            x_t[0][ic][:, t0 + kk:t0 + kk + tn],
                            start=(ic == 0 and kk == 0),
                            stop=(ic == NIC - 1 and kk == KW - 1),
                        )
            for gi, (oc, t0, tn) in enumerate(groups):
                nc.vector.tensor_tensor(
                    dst_l[oc][:, t0:t0 + tn], tiles[gi][:, :tn],
                    bias_sb[:, oc:oc + 1].broadcast_to([128, tn]), op=Add,
                )

        def conv_v_b0_icmajor():
            tiles = [pconv8.tile([128, 512], F32, tag="pc8",
                                 name=f"pc8_v{gi}")
                     for gi in range(len(TKC))]
            for ic in range(NIC):
                for kk in range(KW):
                    for gi, (t0, tn) in enumerate(TKC):
                        nc.tensor.matmul(
                            tiles[gi][:tn, :],
                            x_t[0][ic][:, t0 + kk:t0 + kk + tn],
                            wv_sb[:, ic, kk, :],
                            start=(ic == 0 and kk == 0),
                            stop=(ic == NIC - 1 and kk == KW - 1),
                        )
            for gi, (t0, tn) in enumerate(TKC):
                nc.vector.tensor_tensor(
                    v_tiles[0][:tn, gi, :, DH:2 * DH],
                    tiles[gi][:tn].rearrange("p (h d) -> p h d", h=H),
                    bv_sb[:tn].rearrange("p (h d) -> p h d", h=H),
                    op=Add,
                )

        if IC_MAJOR:
            conv_qk_b0_icmajor("q")
            conv_qk_b0_icmajor("k")
            v_alloc(0)
            conv_v_b0_icmajor()
        else:
            groups = [(oc, t0, tn) for oc in range(NIC) for (t0, tn) in TQ]
            for nm in ("q", "k"):
                dst_l = q_oc if nm == "q" else k_oc
                bias_sb = bq_sb if nm == "q" else bk_sb
                for (oc, t0, tn) in groups:
                    ps = pconv8.tile([128, 512], F32, tag="pc8", name="pc8")
                    mm = 0
                    for ic in range(NIC):
                        for kk in range(KW):
                            nc.tensor.matmul(
                                ps[:, :tn],
                                w_slice(nm, ic, kk,
                                        slice(oc * 128, (oc + 1) * 128)),
                                x_t[0][ic][:, t0 + kk:t0 + kk + tn],
                                start=(mm == 0), stop=(mm == NIC * KW - 1),
                            )
                            mm += 1
                    nc.vector.tensor_tensor(
                        dst_l[oc][:, t0:t0 + tn], ps[:, :tn],
                        bias_sb[:, oc:oc + 1].broadcast_to([128, tn]), op=Add,
                    )
            v_alloc(0)
            for gi, (t0, tn) in enumerate(TKC):
                ps = pconv8.tile([128, 512], F32, tag="pc8", name="pc8")
                mm = 0
                for ic in range(NIC):
                    for kk in range(KW):
                        nc.tensor.matmul(
                            ps[:tn, :],
                            x_t[0][ic][:, t0 + kk:t0 + kk + tn],
                            wv_sb[:, ic, kk, :],
                            start=(mm == 0), stop=(mm == NIC * KW - 1),
                        )
                        mm += 1
                nc.vector.tensor_tensor(
                    v_tiles[0][:tn, gi, :, DH:2 * DH],
                    ps[:tn].rearrange("p (h d) -> p h d", h=H),
                    bv_sb[:tn].rearrange("p (h d) -> p h d", h=H),
                    op=Add,
                )

    if two_phase:
        with tc.tile_pool(name="pconv8", bufs=8, space="PSUM") as pconv8:
            conv_b0_with_pool(pconv8)
        psum_pools["pconv"] = ctx.enter_context(
            tc.tile_pool(name="pconv", bufs=2, space="PSUM"))
        psum_pools["pscore"] = ctx.enter_context(
            tc.tile_pool(name="pscore", bufs=2, space="PSUM"))
        psum_pools["pav"] = ctx.enter_context(
            tc.tile_pool(name="pav", bufs=2, space="PSUM"))
    else:
        for oc in range(NIC):
            conv_qk_piece(0, "q", oc)
        for oc in range(NIC):
            conv_qk_piece(0, "k", oc)
        v_alloc(0)
        for tci in range(len(TKC)):
            conv_v_piece(0, tci)

    # attention b=0 with conv b=1 injected between pairs (fills PE while
    # the ACT engine streams exps; evictions wait on b=0 reads per-tile)
    def inject(h):
        if h == 0:
            v_alloc(1)
            for tci in range(4):
                conv_v_piece(1, tci)
        elif h == 1:
            for tci in range(4, 8):
                conv_v_piece(1, tci)
        elif h in (2, 3, 4):
            # q_oc[oc]/k_oc[oc] are read by scores(0, 2oc) and scores(0, 2oc+1);
            # scores(0, j) is emitted at h = j-1, so conv(1, oc) may only be
            # emitted at h >= 2oc (oc=2 lands exactly at its boundary).
            oc = h - 2
            conv_qk_piece(1, "q", oc)
            conv_qk_piece(1, "k", oc)
        elif h == 5:
            # pull b1's first score pair forward so ACT has exp work
            # queued before the conv filler runs out
            scores(1, 0)
        elif h == 6:
            conv_qk_piece(1, "q", 3)
            conv_qk_piece(1, "k", 3)
            scores(1, 1)

    scores(0, 0)
    for h in range(H):
        if h + 1 < H:
            scores(0, h + 1)
        av(0, h)
        inject(h)

    for h in range(H):
        if h + 1 < H and (1, h + 1) not in pt_tiles:
            scores(1, h + 1)
        av(1, h)

    if loop_n is not None:
        loop_cm.__exit__(None, None, None)
    ctx.close()


_CACHE = {}


def _build(loop_n=None):
    key = ("nc", loop_n)
    if key in _CACHE:
        return _CACHE[key]
    nc = bacc.Bacc("TRN2", target_bir_lowering=False, debug=False,
                   num_devices=NCORES)
    xs = nc.dram_tensor("xs", [BLOC, D, L], MMDT, kind="ExternalInput").ap()
    wq = nc.dram_tensor("wqt", [128, NIC, KW, D], MMDT, kind="ExternalInput").ap()
    wk = nc.dram_tensor("wkt", [128, NIC, KW, D], MMDT, kind="ExternalInput").ap()
    wv = nc.dram_tensor("wvt", [128, NIC, KW, D], MMDT, kind="ExternalInput").ap()
    bq = nc.dram_tensor("bq", [128, NIC], F32, kind="ExternalInput").ap()
    bk = nc.dram_tensor("bk", [128, NIC], F32, kind="ExternalInput").ap()
    bv = nc.dram_tensor("bv", [128, D], F32, kind="ExternalInput").ap()
    out = nc.dram_tensor("out", [BLOC, D, LP], F32, kind="ExternalOutput").ap()
    with tile.TileContext(nc) as tc:
        _emit(tc, xs, wq, wk, wv, bq, bk, bv, out, loop_n=loop_n)
    nc.compile()
    _CACHE[key] = nc
    return nc


def _wt(w):
    # w: [O, I, K] -> [p, ic, k, o] with i = ic*128 + p
    return np.ascontiguousarray(
        w.transpose(1, 2, 0).reshape(NIC, 128, KW, D).transpose(1, 0, 2, 3)
    ).astype(MMDT_NP)


def _in_common(w0, b0, w1, b1, w2, b2):
    return {
        "wqt": _wt(np.asarray(w0, np.float32)),
        "wkt": _wt(np.asarray(w1, np.float32)),
        "wvt": _wt(np.asarray(w2, np.float32)),
        # per-partition bias layouts: [p, oc] with o = oc*128+p
        "bq": np.ascontiguousarray(
            np.asarray(b0, np.float32).reshape(NIC, 128).T),
        "bk": np.ascontiguousarray(
            np.asarray(b1, np.float32).reshape(NIC, 128).T),
        "bv": np.ascontiguousarray(
            np.tile(np.asarray(b2, np.float32)[None, :], (128, 1))),
    }


def kernel(x, w0, b0, w1, b1, w2, b2):
    x = np.asarray(x, dtype=np.float32)
    inp_common = _in_common(w0, b0, w1, b1, w2, b2)
    nc = _build()
    in_maps = [
        {"xs": np.ascontiguousarray(x[c * BLOC:(c + 1) * BLOC]).astype(MMDT_NP), **inp_common}
        for c in range(NCORES)
    ]
    res = run_bass_kernel_spmd(nc, in_maps, list(range(NCORES)))
    return np.concatenate([res.results[c]["out"] for c in range(NCORES)], axis=0)


def run_traced(x, w0, b0, w1, b1, w2, b2, **kw):
    """Like kernel() but returns (output, BassKernelResults)."""
    x = np.asarray(x, dtype=np.float32)
    inp_common = _in_common(w0, b0, w1, b1, w2, b2)
    nc = _build()
    in_maps = [
        {"xs": np.ascontiguousarray(x[c * BLOC:(c + 1) * BLOC]).astype(MMDT_NP), **inp_common}
        for c in range(NCORES)
    ]
    res = run_bass_kernel_spmd(nc, in_maps, list(range(NCORES)), **kw)
    out = np.concatenate([res.results[c]["out"] for c in range(NCORES)], axis=0)
    return out, res



# revision 13
# speedup vs baseline: 1.2846x; 1.0674x over previous
"""Trainium2 Bass kernel: ConvolutionalMultiheadAttention.

Reference computation (per batch element b):
    q = conv1d(x, w0) + b0          # [D, Lp]  (VALID, K=3)
    k = conv1d(x, w1) + b1
    v = conv1d(x, w2) + b2
    per head h (Dh=64): out_h = v_h @ softmax(q_h^T k_h / sqrt(D))^T

Sharding: data-parallel over batch B=16 across 8 cores (2 per core).
Weights replicated. No collectives.

Per-core kernel architecture (PE-roofline oriented; all matmuls stream
~2 rows/cycle in bf16/fp16):
  - conv as matmul: contraction over input channel i (4 chunks of 128),
    accumulating 4*3 = 12 matmuls per PSUM tile. q,k produced in
    [o_part, t_free] layout; v produced transposed [t_part, o_free]
    (lhsT = x slice, rhs = w2 slice).
  - v tile layout [t_part, ktc, h, 128]: columns 0..63 are constant 1.0,
    columns 64..127 hold v. The AV matmul then yields the softmax
    denominator on PSUM partition 0 (a legal base for
    reciprocal_approx_fast) and the output block on partitions 64..127
    (PSUM reads must be 0/64-aligned).
  - scores computed transposed: S_T[kt, qt] = k_h^T q_h (contraction
    over d=64 on partitions). exp via ACT engine with the 1/sqrt(512)
    scale folded into the activation, output in fp16 (P_T).
  - AV: accumulate [1|v]^T @ P_T over kt chunks; normalize via
    reciprocal_approx_fast (DVE) + partition_broadcast (GPSIMD) +
    multiply (DVE), DMA straight to the output in [o, t] layout.
  - scheduling: scores(0,0..3) are interleaved INTO the b0 conv phase so
    the ACT engine (142us of exp work) starts ~40us earlier; a unified
    16-slot (b,h) loop then emits scores with a 3-head lookahead cursor
    while b1 conv pieces are injected across slots 2..9, keeping the PE
    busy while ACT streams exps. pt pool holds up to 4 heads of P.
  - input DMA issue is spread across the three DMA-capable engines
    (sync/SP, scalar/ACT, gpsimd) because each dma_start costs ~600ns of
    serial descriptor-write time on its issuing engine; the first conv
    matmul is gated only on a 128KB weight slice + half an x chunk.
"""

import numpy as np

import concourse.bass as bass
import concourse.bacc as bacc
import concourse.mybir as mybir
import concourse.tile as tile
from concourse.bass_utils import run_bass_kernel_spmd

B, D, L, KW, H = 16, 512, 1024, 3, 8
LP = L - KW + 1          # 1022
DH = D // H              # 64
NCORES = 8
BLOC = B // NCORES       # 2
NIC = D // 128           # 4 input-channel chunks
SCALE = 1.0 / float(np.sqrt(D))
import os
MM_DTYPE_NAME = os.environ.get('MM_DTYPE', 'bf16')

F32 = mybir.dt.float32
F32R = mybir.dt.float32r
F16 = mybir.dt.float16
BF16 = mybir.dt.bfloat16
MMDT = {"f32r": F32R, "bf16": BF16, "f32": F32}[MM_DTYPE_NAME]
import ml_dtypes
MMDT_NP = {"f32r": np.float32, "bf16": ml_dtypes.bfloat16, "f32": np.float32}[MM_DTYPE_NAME]

# time chunking
TQ = [(0, 512), (512, LP - 512)]                       # qt chunks (512, 510)
TKC = [(i * 128, min(128, LP - i * 128)) for i in range(8)]  # kt chunks (...126)

PT_BUFS = int(os.environ.get('PT_BUFS', '32'))
LOOKAHEAD = int(os.environ.get('LOOKAHEAD', '3'))


def _emit(tc, xs, wq, wk, wv, bq, bk, bv, out):
    nc = tc.nc
    Exp = mybir.ActivationFunctionType.Exp
    from concourse.alu_op_type import AluOpType
    Add = AluOpType.add
    from contextlib import ExitStack
    ctx = ExitStack()
    wpool = ctx.enter_context(tc.tile_pool(name="w", bufs=1))
    cpool = ctx.enter_context(tc.tile_pool(name="const", bufs=1))
    xpool = ctx.enter_context(tc.tile_pool(name="x", bufs=1))
    qkpool = ctx.enter_context(tc.tile_pool(name="qk", bufs=1))
    vpool = ctx.enter_context(tc.tile_pool(name="v", bufs=2))
    ptpool = ctx.enter_context(tc.tile_pool(name="pt", bufs=PT_BUFS))
    opool = ctx.enter_context(tc.tile_pool(name="o", bufs=3))
    rpool = ctx.enter_context(tc.tile_pool(name="r", bufs=2))
    bpool = ctx.enter_context(tc.tile_pool(name="bc", bufs=2))
    # PSUM (8 banks): conv-b0 phase: pconv8 (4 banks) + pscore (2x2
    # banks); after: pconv (2) + pav (2) + pscore (4).
    psum_pools = {}
    psum_pools["pscore"] = ctx.enter_context(
        tc.tile_pool(name="pscore", bufs=2, space="PSUM"))

    # ---- loads: spread descriptor-issue across sync/scalar/gpsimd and
    # order by first-need time ----
    wq_ic = []
    wk_ic = []
    x_t = [[None] * NIC for _ in range(BLOC)]
    x0r = xs[0].rearrange("(c p) t -> p c t", p=128)
    x1r = xs[1].rearrange("(c p) t -> p c t", p=128)

    # sync: q/k weights, interleaved in conv consumption order
    for ic in range(NIC):
        t = wpool.tile([128, KW, D], MMDT, tag=f"wq{ic}", name=f"wq{ic}")
        if ic == 0:
            for kk in range(KW):
                nc.sync.dma_start(t[:, kk], wq[:, ic, kk])
        else:
            nc.sync.dma_start(t[:], wq[:, ic])
        wq_ic.append(t)
        t = wpool.tile([128, KW, D], MMDT, tag=f"wk{ic}", name=f"wk{ic}")
        nc.sync.dma_start(t[:], wk[:, ic])
        wk_ic.append(t)

    # scalar (ACT): x for b0 (gates the first conv matmuls; exps queue
    # behind these and start much later)
    for ic in range(NIC):
        xt = xpool.tile([128, L], MMDT, tag=f"x0{ic}", name=f"x0{ic}")
        if ic == 0:
            nc.scalar.dma_start(xt[:, 0:516], x0r[:, ic, 0:516])
            nc.scalar.dma_start(xt[:, 516:L], x0r[:, ic, 516:L])
        else:
            nc.scalar.dma_start(xt[:], x0r[:, ic])
        x_t[0][ic] = xt

    # gpsimd: biases, v weights, x for b1 (needed tens of us in)
    bq_sb = cpool.tile([128, NIC], F32, tag="bq")
    nc.gpsimd.dma_start(bq_sb[:], bq[:])
    bk_sb = cpool.tile([128, NIC], F32, tag="bk")
    nc.gpsimd.dma_start(bk_sb[:], bk[:])
    wv_sb = wpool.tile([128, NIC, KW, D], MMDT, tag="wv")
    nc.gpsimd.dma_start(wv_sb[:], wv[:])
    bv_sb = cpool.tile([128, D], F32, tag="bv")
    nc.gpsimd.dma_start(bv_sb[:], bv[:])
    for ic in range(NIC):
        xt = xpool.tile([128, L], MMDT, tag=f"x1{ic}", name=f"x1{ic}")
        nc.gpsimd.dma_start(xt[:], x1r[:, ic])
        x_t[1][ic] = xt

    def w_slice(nm, ic, kk, osl):
        if nm == "q":
            return wq_ic[ic][:, kk, osl]
        if nm == "k":
            return wk_ic[ic][:, kk, osl]
        return wv_sb[:, ic, kk, osl]

    # q/k: per-oc tiles [p, t] with o = oc*128+p (reused in-place across b)
    q_oc = [qkpool.tile([128, L], MMDT, tag=f"q{oc}", name=f"q{oc}") for oc in range(NIC)]
    k_oc = [qkpool.tile([128, L], MMDT, tag=f"k{oc}", name=f"k{oc}") for oc in range(NIC)]
    v_tiles = [None, None]

    def conv_qk_piece(b, nm, oc, pool):
        dst = (q_oc if nm == "q" else k_oc)[oc]
        bias_sb = bq_sb if nm == "q" else bk_sb
        for (t0, tn) in TQ:
            ps = pool.tile([128, 512], F32, tag="pc", name="pc")
            mm = 0
            for ic in range(NIC):
                for kk in range(KW):
                    nc.tensor.matmul(
                        ps[:, :tn],
                        w_slice(nm, ic, kk, slice(oc * 128, (oc + 1) * 128)),
                        x_t[b][ic][:, t0 + kk:t0 + kk + tn],
                        start=(mm == 0), stop=(mm == NIC * KW - 1),
                    )
                    mm += 1
            nc.vector.tensor_tensor(
                dst[:, t0:t0 + tn], ps[:, :tn],
                bias_sb[:, oc:oc + 1].broadcast_to([128, tn]), op=Add,
            )

    def v_alloc(b):
        v_sb = vpool.tile([128, 8, H, 2 * DH], F16, tag="v")
        nc.vector.memset(v_sb[:, :, :, 0:DH], 1.0)
        v_tiles[b] = v_sb

    def conv_v_piece(b, tci, pool):
        t0, tn = TKC[tci]
        ps = pool.tile([128, 512], F32, tag="pc", name="pc")
        mm = 0
        for ic in range(NIC):
            for kk in range(KW):
                nc.tensor.matmul(
                    ps[:tn, :],
                    x_t[b][ic][:, t0 + kk:t0 + kk + tn],
                    wv_sb[:, ic, kk, :],
                    start=(mm == 0), stop=(mm == NIC * KW - 1),
                )
                mm += 1
        nc.vector.tensor_tensor(
            v_tiles[b][:tn, tci, :, DH:2 * DH],
            ps[:tn].rearrange("p (h d) -> p h d", h=H),
            bv_sb[:tn].rearrange("p (h d) -> p h d", h=H),
            op=Add,
        )

    # ---- attention ----
    pt_tiles = {}

    def scores(b, h):
        po = 64 * (h % 2)
        oc = h // 2
        kh = k_oc[oc][po:po + DH, :]
        qh = q_oc[oc][po:po + DH, :]
        tiles = []
        for (kt0, ktn) in TKC:
            pt = ptpool.tile([128, L], F16, tag="pt")
            ss = psum_pools["pscore"].tile([128, 1024], F32, tag="ps", name="ss")
            for (qt0, qtn) in TQ:
                nc.tensor.matmul(
                    ss[:ktn, qt0:qt0 + qtn],
                    kh[:, kt0:kt0 + ktn],
                    qh[:, qt0:qt0 + qtn],
                    start=True, stop=True,
                )
            nc.scalar.activation(pt[:ktn, 0:LP], ss[:ktn, 0:LP], Exp,
                                 scale=SCALE)
            tiles.append(pt)
        pt_tiles[(b, h)] = tiles

    def av(b, h):
        tiles = pt_tiles.pop((b, h))
        for (qt0, qtn) in TQ:
            pa = psum_pools["pav"].tile([128, 512], F32, tag="pa", name="pa")
            for tci, (kt0, ktn) in enumerate(TKC):
                nc.tensor.matmul(
                    pa[:2 * DH, :qtn],
                    v_tiles[b][:ktn, tci, h, :],
                    tiles[tci][:ktn, qt0:qt0 + qtn],
                    start=(tci == 0), stop=(tci == len(TKC) - 1),
                )
            rec = rpool.tile([1, 512], F32, tag="rec")
            nc.vector.reciprocal_approx_fast(rec[:1, :qtn], pa[0:1, :qtn])
            brd = bpool.tile([DH, 512], F32, tag="brd")
            nc.gpsimd.partition_broadcast(brd[:, :qtn], rec[:1, :qtn])
            ot = opool.tile([DH, 512], F32, tag="ot")
            nc.vector.tensor_mul(ot[:, :qtn], pa[DH:2 * DH, :qtn], brd[:, :qtn])
            nc.sync.dma_start(
                out[b, DH * h:DH * (h + 1), qt0:qt0 + qtn], ot[:, :qtn]
            )

    # ---- phase 1: b0 conv with scores(0,0..3) interleaved (ACT starts
    # its 142us of exp work ~40us earlier than a conv-then-attention
    # schedule; consecutive scores calls are spaced ~10us of conv so the
    # 2-deep pscore ring never backs up into the PE) ----
    with tc.tile_pool(name="pconv8", bufs=4, space="PSUM") as p8:
        conv_qk_piece(0, "q", 0, p8)
        conv_qk_piece(0, "k", 0, p8)
        conv_qk_piece(0, "q", 1, p8)
        conv_qk_piece(0, "k", 1, p8)
        scores(0, 0)
        conv_qk_piece(0, "q", 2, p8)
        conv_qk_piece(0, "k", 2, p8)
        scores(0, 1)
        conv_qk_piece(0, "q", 3, p8)
        conv_qk_piece(0, "k", 3, p8)
        scores(0, 2)
        v_alloc(0)
        for tci in range(4):
            conv_v_piece(0, tci, p8)
        scores(0, 3)
        for tci in range(4, 8):
            conv_v_piece(0, tci, p8)

    psum_pools["pconv"] = ctx.enter_context(
        tc.tile_pool(name="pconv", bufs=2, space="PSUM"))
    psum_pools["pav"] = ctx.enter_context(
        tc.tile_pool(name="pav", bufs=2, space="PSUM"))

    # ---- phase 2: unified 16-slot loop over (b,h); scores cursor runs
    # LOOKAHEAD heads ahead; b1 conv pieces injected to keep the PE fed
    # while ACT streams exps ----
    HEADS = [(0, h) for h in range(H)] + [(1, h) for h in range(H)]
    pconv = psum_pools["pconv"]
    INJ = {
        2: [lambda: v_alloc(1),
            lambda: conv_v_piece(1, 0, pconv),
            lambda: conv_v_piece(1, 1, pconv)],
        3: [lambda: conv_qk_piece(1, "q", 0, pconv),
            lambda: conv_qk_piece(1, "k", 0, pconv)],
        4: [lambda: conv_v_piece(1, 2, pconv),
            lambda: conv_v_piece(1, 3, pconv)],
        5: [lambda: conv_qk_piece(1, "q", 1, pconv),
            lambda: conv_qk_piece(1, "k", 1, pconv)],
        6: [lambda: conv_v_piece(1, 4, pconv),
            lambda: conv_v_piece(1, 5, pconv),
            lambda: conv_v_piece(1, 6, pconv),
            lambda: conv_v_piece(1, 7, pconv)],
        7: [lambda: conv_qk_piece(1, "q", 2, pconv),
            lambda: conv_qk_piece(1, "k", 2, pconv)],
        9: [lambda: conv_qk_piece(1, "q", 3, pconv),
            lambda: conv_qk_piece(1, "k", 3, pconv)],
    }
    # earliest slot at which scores(1,h) may be emitted (its q/k conv
    # piece must have been injected in a previous slot)
    MIN_SLOT = {h: {0: 4, 1: 4, 2: 6, 3: 6, 4: 8, 5: 8, 6: 10, 7: 10}[h]
                for h in range(H)}

    cursor = 4  # scores(0,0..3) already emitted in phase 1
    for i, (b, h) in enumerate(HEADS):
        while cursor < len(HEADS) and cursor <= i + LOOKAHEAD:
            cb, ch = HEADS[cursor]
            if cb == 1 and i < MIN_SLOT[ch]:
                break
            scores(cb, ch)
            cursor += 1
        av(b, h)
        for fn in INJ.get(i, ()):
            fn()
    assert cursor == len(HEADS)
    ctx.close()


_CACHE = {}


def _build():
    key = "nc"
    if key in _CACHE:
        return _CACHE[key]
    nc = bacc.Bacc("TRN2", target_bir_lowering=False, debug=False,
                   num_devices=NCORES)
    xs = nc.dram_tensor("xs", [BLOC, D, L], MMDT, kind="ExternalInput").ap()
    wq = nc.dram_tensor("wqt", [128, NIC, KW, D], MMDT, kind="ExternalInput").ap()
    wk = nc.dram_tensor("wkt", [128, NIC, KW, D], MMDT, kind="ExternalInput").ap()
    wv = nc.dram_tensor("wvt", [128, NIC, KW, D], MMDT, kind="ExternalInput").ap()
    bq = nc.dram_tensor("bq", [128, NIC], F32, kind="ExternalInput").ap()
    bk = nc.dram_tensor("bk", [128, NIC], F32, kind="ExternalInput").ap()
    bv = nc.dram_tensor("bv", [128, D], F32, kind="ExternalInput").ap()
    out = nc.dram_tensor("out", [BLOC, D, LP], F32, kind="ExternalOutput").ap()
    with tile.TileContext(nc) as tc:
        _emit(tc, xs, wq, wk, wv, bq, bk, bv, out)
    nc.compile()
    _CACHE[key] = nc
    return nc


def _wt(w):
    # w: [O, I, K] -> [p, ic, k, o] with i = ic*128 + p
    return np.ascontiguousarray(
        w.transpose(1, 2, 0).reshape(NIC, 128, KW, D).transpose(1, 0, 2, 3)
    ).astype(MMDT_NP)


def _in_common(w0, b0, w1, b1, w2, b2):
    return {
        "wqt": _wt(np.asarray(w0, np.float32)),
        "wkt": _wt(np.asarray(w1, np.float32)),
        "wvt": _wt(np.asarray(w2, np.float32)),
        # per-partition bias layouts: [p, oc] with o = oc*128+p
        "bq": np.ascontiguousarray(
            np.asarray(b0, np.float32).reshape(NIC, 128).T),
        "bk": np.ascontiguousarray(
            np.asarray(b1, np.float32).reshape(NIC, 128).T),
        "bv": np.ascontiguousarray(
            np.tile(np.asarray(b2, np.float32)[None, :], (128, 1))),
    }


def kernel(x, w0, b0, w1, b1, w2, b2):
    x = np.asarray(x, dtype=np.float32)
    inp_common = _in_common(w0, b0, w1, b1, w2, b2)
    nc = _build()
    in_maps = [
        {"xs": np.ascontiguousarray(x[c * BLOC:(c + 1) * BLOC]).astype(MMDT_NP), **inp_common}
        for c in range(NCORES)
    ]
    res = run_bass_kernel_spmd(nc, in_maps, list(range(NCORES)))
    return np.concatenate([res.results[c]["out"] for c in range(NCORES)], axis=0)


def run_traced(x, w0, b0, w1, b1, w2, b2, **kw):
    """Like kernel() but returns (output, BassKernelResults)."""
    x = np.asarray(x, dtype=np.float32)
    inp_common = _in_common(w0, b0, w1, b1, w2, b2)
    nc = _build()
    in_maps = [
        {"xs": np.ascontiguousarray(x[c * BLOC:(c + 1) * BLOC]).astype(MMDT_NP), **inp_common}
        for c in range(NCORES)
    ]
    res = run_bass_kernel_spmd(nc, in_maps, list(range(NCORES)), **kw)
    out = np.concatenate([res.results[c]["out"] for c in range(NCORES)], axis=0)
    return out, res
